# revision 1
# baseline (speedup 1.0000x reference)
"""DeepSeekV3-style MoE layer (1 MoE block) on 8 Trainium2 NeuronCores.

Sharding: expert-parallel. Each core owns 4 of the 32 routed experts and a
64-wide shard of the shared expert's intermediate dim. The router is
replicated (router weight columns are permuted per-core so the local experts
always sit in columns 0..3 — top-k and sigmoid are permutation invariant).
Partial outputs are combined with per-chunk on-device ReduceScatters that
overlap compute; the host reassembles the 8 output shards.

Per-core device pipeline (feature-major activations):
  - x is cast to bf16 (plus a bf16 residual x - bf16(x)) token-major, bounced
    through DRAM, and loaded back transposed via DMA-transpose -> xT tiles
  - router runs in split-bf16 with ~fp32 accuracy:
    logits = w1.x1 + w1.x2 + w2.x1 accumulated in one fp32 PSUM group
  - top-8 selection on logits via iterative max extraction (sigmoid is
    monotonic so logit order == affinity order), normalized sigmoid weights
  - bf16 gate/up matmuls -> silu(g+bg) * (u+bu) * token_weight -> bf16 hge
  - down-projection with hge as the stationary operand so the PSUM output is
    token-major [128 tokens x H], accumulating all 4 experts + shared expert
    + bias trick ([w_e rows; ones] @ [bd_e rows; bd_shared]) in one group.
"""

import sys

sys.path.insert(0, "/opt/trn_rl_repo")

import numpy as np

import concourse.bacc as bacc
import concourse.bass as bass
import concourse.mybir as mybir
import concourse.tile as tile
from concourse.masks import make_identity

F32 = mybir.dt.float32
BF16 = mybir.dt.bfloat16
AF = mybir.ActivationFunctionType
ALU = mybir.AluOpType

H, I, E, TOPK = 1024, 512, 32, 8
B, S = 4, 1024
T = B * S
NCORES = 8
E_LOC = E // NCORES          # 4 routed experts per core
I_SH = I // NCORES           # 64-wide shared-expert shard per core
P = 128
TC = 512                     # token chunk
NCH = T // TC                # 8 chunks
NH = H // P                  # 8 hidden k-tiles
NI = I // P                  # 4 intermediate tiles
NJ = TC // P                 # 4 token tiles per chunk
T_SHARD = T // NCORES        # 512 rows per core after ReduceScatter
RS_SH = TC // NCORES         # 64 rows per core per chunk ReduceScatter
NEG = -1.0e30


def build_nc():
    nc = bacc.Bacc(None, target_bir_lowering=False, num_devices=NCORES)

    x_d = nc.declare_dram_parameter("x", [T, H], F32, isOutput=False)
    wr_d = nc.declare_dram_parameter("wr", [H, E], F32, isOutput=False)
    br_d = nc.declare_dram_parameter("br", [E], F32, isOutput=False)
    wg_d = nc.declare_dram_parameter("wg", [E_LOC, H, I], F32, isOutput=False)
    wu_d = nc.declare_dram_parameter("wu", [E_LOC, H, I], F32, isOutput=False)
    wd_d = nc.declare_dram_parameter("wd", [E_LOC, I, H], F32, isOutput=False)
    bg_d = nc.declare_dram_parameter("bg", [E_LOC, I], F32, isOutput=False)
    bu_d = nc.declare_dram_parameter("bu", [E_LOC, I], F32, isOutput=False)
    bias5_d = nc.declare_dram_parameter("bias5", [E_LOC + 1, H], F32, isOutput=False)
    wgs_d = nc.declare_dram_parameter("wgs", [H, I_SH], F32, isOutput=False)
    wus_d = nc.declare_dram_parameter("wus", [H, I_SH], F32, isOutput=False)
    wds_d = nc.declare_dram_parameter("wds", [I_SH, H], F32, isOutput=False)
    bgs_d = nc.declare_dram_parameter("bgs", [I_SH], F32, isOutput=False)
    bus_d = nc.declare_dram_parameter("bus", [I_SH], F32, isOutput=False)
    sel_d = nc.declare_dram_parameter("sel", [E_LOC, E_LOC * P], F32, isOutput=False)
    y_d = nc.declare_dram_parameter("y", [T_SHARD, H], F32, isOutput=True)

    cc_in = nc.dram_tensor("cc_in", [T, H], F32)
    cc_out = nc.dram_tensor("cc_out", [T_SHARD, H], F32)
    xbf_dram = nc.dram_tensor("xbf_dram", [T, H], BF16)
    xr_dram = nc.dram_tensor("xr_dram", [T, H], BF16)

    with tile.TileContext(nc) as tc:
        with (
            tc.tile_pool(name="wres", bufs=1) as wres,
            tc.tile_pool(name="xp", bufs=2) as xp,
            tc.tile_pool(name="xtb", bufs=2) as xtb,
            tc.tile_pool(name="xtb2", bufs=2) as xtb2,
            tc.tile_pool(name="hgep", bufs=1) as hgep,
            tc.tile_pool(name="actp", bufs=2) as actp,
            tc.tile_pool(name="outp", bufs=2) as outp,
            tc.tile_pool(name="rtp", bufs=2) as rtp,
            tc.tile_pool(name="wstg", bufs=2) as wstg,
            tc.tile_pool(name="ps_tr", bufs=1, space="PSUM") as ps_tr,
            tc.tile_pool(name="ps_r", bufs=1, space="PSUM") as ps_r,
            tc.tile_pool(name="ps_g", bufs=2, space="PSUM") as ps_g,
            tc.tile_pool(name="ps_u", bufs=2, space="PSUM") as ps_u,
            tc.tile_pool(name="ps_d", bufs=1, space="PSUM") as ps_d,
        ):
            # ---------- constants / small weights ----------
            ident = wres.tile([P, P], F32, tag="ident")
            make_identity(nc, ident[:])

            def stage_x(ch):
                """DMA x chunk, cast to bf16 + residual, bounce via DRAM,
                load back transposed."""
                t0 = ch * TC
                for j in range(NJ):
                    r0 = t0 + j * P
                    xtm = xp.tile([P, H], F32, tag="xtm")
                    nc.sync.dma_start(xtm[:], x_d[r0:r0 + P, :])
                    x_bf = xp.tile([P, H], BF16, tag="x_bf")
                    nc.vector.tensor_copy(x_bf[:], xtm[:])
                    x_r = xp.tile([P, H], BF16, tag="x_r")
                    nc.vector.tensor_tensor(x_r[:], xtm[:], x_bf[:], ALU.subtract)
                    nc.sync.dma_start(xbf_dram[r0:r0 + P, :], x_bf[:])
                    nc.sync.dma_start(xr_dram[r0:r0 + P, :], x_r[:])
                xtb_t = {}
                for h in range(NH):
                    xt = xtb.tile([P, TC], BF16, tag=f"xtb{h}", name=f"xtb{h}")
                    nc.sync.dma_start_transpose(
                        xt[:], xbf_dram[t0:t0 + TC, h * P:(h + 1) * P])
                    xtb_t[h] = xt
                return xtb_t

            def load_xt2(ch, h):
                t0 = ch * TC
                xt2 = xtb2.tile([P, TC], BF16, tag="xt2", name="xt2")
                nc.sync.dma_start_transpose(
                    xt2[:], xr_dram[t0:t0 + TC, h * P:(h + 1) * P])
                return xt2

            # chunk 0 x pipeline first so PE work is unblocked early
            xtb_chunks = {0: stage_x(0)}

            # router weights: split-bf16 pair w1 + w2 ~= Wr (fp32)
            wr1_sb = {}
            wr2_sb = {}
            for h in range(NH):
                st = actp.tile([P, E], F32, tag="stage_s", name="strr")
                nc.sync.dma_start(st[:], wr_d[h * P:(h + 1) * P, :])
                w1 = wres.tile([P, E], BF16, tag=f"wr1_{h}", name="wr1")
                nc.vector.tensor_copy(w1[:], st[:])
                w2 = wres.tile([P, E], BF16, tag=f"wr2_{h}", name="wr2")
                nc.vector.tensor_tensor(w2[:], st[:], w1[:], ALU.subtract)
                wr1_sb[h] = w1
                wr2_sb[h] = w2

            # selector constant for per-expert weight-row broadcast
            sel_bf = wres.tile([E_LOC, E_LOC * P], BF16, tag="sel")
            nc.gpsimd.dma_start(sel_bf[:], sel_d[:])

            # biases
            br_sb = wres.tile([E, 1], F32, tag="br")
            nc.sync.dma_start(br_sb[:], br_d.rearrange("(e o) -> e o", o=1))
            bg_sb = wres.tile([P, E_LOC, NI], F32, tag="bg")
            nc.sync.dma_start(bg_sb[:], bg_d.rearrange("e (i p) -> p e i", p=P))
            bu_sb = wres.tile([P, E_LOC, NI], F32, tag="bu")
            nc.sync.dma_start(bu_sb[:], bu_d.rearrange("e (i p) -> p e i", p=P))
            bgs_sb = wres.tile([I_SH, 1], F32, tag="bgs")
            nc.sync.dma_start(bgs_sb[:], bgs_d.rearrange("(e o) -> e o", o=1))
            bus_sb = wres.tile([I_SH, 1], F32, tag="bus")
            nc.sync.dma_start(bus_sb[:], bus_d.rearrange("(e o) -> e o", o=1))
            bias5_sb = wres.tile([E_LOC + 1, H], BF16, tag="bias5")
            nc.gpsimd.dma_start(bias5_sb[:], bias5_d[:])

            # routing weights, feature-major: rows 0..3 local expert w, row 4 ones
            we_sb = wres.tile([E_LOC + 1, T], BF16, tag="we")
            nc.vector.memset(we_sb[:], 1.0)

            def router(ch, xtb_t):
                t0 = ch * TC
                pr = ps_r.tile([E, TC], F32, tag="r", name="pr")
                for h in range(NH):
                    xt2 = load_xt2(ch, h)
                    nc.tensor.matmul(pr[:], wr1_sb[h][:], xtb_t[h][:],
                                     start=(h == 0), stop=False)
                    nc.tensor.matmul(pr[:], wr1_sb[h][:], xt2[:],
                                     start=False, stop=False)
                    nc.tensor.matmul(pr[:], wr2_sb[h][:], xtb_t[h][:],
                                     start=False, stop=(h == NH - 1))
                logits_fm = rtp.tile([E, TC], F32, tag="logits_fm", bufs=1)
                nc.scalar.activation(logits_fm[:], pr[:], AF.Identity,
                                     bias=br_sb[:, 0:1])
                # transpose to token-major [128, 4, 32]
                logits_tm = rtp.tile([P, NJ, E], F32, tag="logits_tm")
                for j in range(NJ):
                    pt = ps_tr.tile([P, E], F32, tag="tr", name="ptl")
                    nc.tensor.transpose(pt[:], logits_fm[:, j * P:(j + 1) * P],
                                        ident[0:E, 0:E])
                    nc.vector.tensor_copy(logits_tm[:, j, :], pt[:])
                # top-8 threshold by iterative max extraction
                cur = rtp.tile([P, NJ, E], F32, tag="cur")
                nc.vector.tensor_copy(cur[:], logits_tm[:])
                mx = rtp.tile([P, NJ], F32, tag="mx")
                mask = rtp.tile([P, NJ, E], F32, tag="mask", bufs=1)
                for k in range(TOPK):
                    nc.vector.tensor_reduce(mx[:], cur[:], mybir.AxisListType.X,
                                            ALU.max)
                    if k < TOPK - 1:
                        mxb = mx[:].rearrange("p (f o) -> p f o", o=1).broadcast_to(
                            [P, NJ, E])
                        nc.vector.tensor_tensor(mask[:], cur[:], mxb, ALU.is_ge)
                        nc.vector.scalar_tensor_tensor(cur[:], mask[:], NEG, cur[:],
                                                       ALU.mult, ALU.add)
                # mask8 / normalized sigmoid weights
                aff = rtp.tile([P, NJ, E], F32, tag="aff")
                nc.scalar.activation(aff[:], logits_tm[:], AF.Sigmoid)
                thrb = mx[:].rearrange("p (f o) -> p f o", o=1).broadcast_to(
                    [P, NJ, E])
                nc.vector.tensor_tensor(mask[:], logits_tm[:], thrb, ALU.is_ge)
                nc.vector.tensor_tensor(aff[:], aff[:], mask[:], ALU.mult)
                den = rtp.tile([P, NJ], F32, tag="den")
                nc.vector.tensor_reduce(den[:], aff[:], mybir.AxisListType.X, ALU.add)
                rec = rtp.tile([P, NJ], F32, tag="rec")
                nc.vector.reciprocal(rec[:], den[:])
                recb = rec[:].rearrange("p (f o) -> p f o", o=1).broadcast_to(
                    [P, NJ, E])
                w_tm = rtp.tile([P, NJ, E], F32, tag="w_tm")
                nc.vector.tensor_tensor(w_tm[:], aff[:], recb, ALU.mult)
                # local expert weights, feature-major -> we_sb rows 0..3 (bf16)
                for j in range(NJ):
                    pt = ps_tr.tile([E_LOC, P], F32, tag="tr", name="ptw")
                    nc.tensor.transpose(pt[:], w_tm[:, j, 0:E_LOC], ident[:])
                    nc.vector.tensor_copy(
                        we_sb[0:E_LOC, t0 + j * P:t0 + (j + 1) * P], pt[:])

            router(0, xtb_chunks[0])

            # ---------- resident expert weights (fp32 load -> bf16 cast) ----------
            wg_bf = {}
            wu_bf = {}
            wd_bf = {}
            # gate/up: [1024, 512] -> [128, 8, 512] partition-major; 1MB DMAs,
            # DVE casts (needed first). down: GpSimd casts (idle engine).
            for e in range(E_LOC):
                for name, dram, store in (("wg", wg_d, wg_bf), ("wu", wu_d, wu_bf)):
                    res = wres.tile([P, NH, I], BF16, tag=f"{name}{e}", name="wres_gu")
                    for half in range(2):
                        hh = NH // 2
                        st = wstg.tile([P, hh, I], F32, tag="wst", name="stgu")
                        nc.scalar.dma_start(
                            st[:],
                            dram[e].rearrange("(ho p) i -> p ho i", p=P)[
                                :, half * hh:(half + 1) * hh, :])
                        nc.vector.tensor_copy(res[:, half * hh:(half + 1) * hh, :],
                                              st[:])
                    store[e] = res
            wgs_bf = {}
            wus_bf = {}
            for name, dram, store in (("wgs", wgs_d, wgs_bf), ("wus", wus_d, wus_bf)):
                st = wstg.tile([P, NH, I_SH], F32, tag="stage_s", name="sts", bufs=1)
                nc.scalar.dma_start(st[:], dram.rearrange("(ho p) i -> p ho i", p=P))
                res = wres.tile([P, NH, I_SH], BF16, tag=f"{name}", name="wsbf")
                nc.vector.tensor_copy(res[:], st[:])
                store[0] = res
            for e in range(E_LOC):
                res = wres.tile([P, NI, H], BF16, tag=f"wd{e}", name="wres_d")
                for half in range(2):
                    ih = NI // 2
                    st = wstg.tile([P, ih, H], F32, tag="wst", name="std")
                    nc.scalar.dma_start(
                        st[:],
                        wd_d[e].rearrange("(io p) h -> p io h", p=P)[
                            :, half * ih:(half + 1) * ih, :])
                    nc.gpsimd.tensor_copy(res[:, half * ih:(half + 1) * ih, :], st[:])
                wd_bf[e] = res
            st = wstg.tile([I_SH, H], F32, tag="wst", name="stds")
            nc.scalar.dma_start(st[:], wds_d[:])
            wds_bf = wres.tile([I_SH, H], BF16, tag="wds")
            nc.gpsimd.tensor_copy(wds_bf[:], st[:])

            def experts(ch, xtb_t):
                t0 = ch * TC
                # gate/up -> hge (bf16)
                hge = {}
                for e in range(E_LOC):
                    # broadcast token-weight row -> [128, TC] via selector matmul
                    pw = ps_r.tile([P, TC], F32, tag="r", name="pw")
                    nc.tensor.matmul(pw[:], sel_bf[:, e * P:(e + 1) * P],
                                     we_sb[0:E_LOC, t0:t0 + TC],
                                     start=True, stop=True)
                    w_bc = actp.tile([P, TC], BF16, tag="w_bc", bufs=1)
                    nc.vector.tensor_copy(w_bc[:], pw[:])
                    for i in range(NI):
                        pg = ps_g.tile([P, TC], F32, tag="g")
                        pu = ps_u.tile([P, TC], F32, tag="u")
                        for h in range(NH):
                            nc.tensor.matmul(pg[:],
                                             wg_bf[e][:, h, i * P:(i + 1) * P],
                                             xtb_t[h][:], start=(h == 0),
                                             stop=(h == NH - 1))
                        for h in range(NH):
                            nc.tensor.matmul(pu[:],
                                             wu_bf[e][:, h, i * P:(i + 1) * P],
                                             xtb_t[h][:], start=(h == 0),
                                             stop=(h == NH - 1))
                        g_act = actp.tile([P, TC], F32, tag="g_act")
                        nc.scalar.activation(g_act[:], pg[:], AF.Silu,
                                             bias=bg_sb[:, e, i:i + 1])
                        u_w = actp.tile([P, TC], F32, tag="u_w")
                        nc.vector.scalar_tensor_tensor(
                            u_w[:], pu[:], bu_sb[:, e, i:i + 1], w_bc[:],
                            ALU.add, ALU.mult)
                        ht = hgep.tile([P, TC], BF16, tag=f"hge{e}_{i}", name="ht")
                        nc.vector.tensor_tensor(ht[:], g_act[:], u_w[:], ALU.mult)
                        hge[(e, i)] = ht

                # shared expert shard -> hge_s (bf16, 64 partitions)
                psg = ps_g.tile([I_SH, TC], F32, tag="g", name="psg")
                psu = ps_u.tile([I_SH, TC], F32, tag="u", name="psu")
                for h in range(NH):
                    nc.tensor.matmul(psg[:], wgs_bf[0][:, h, :], xtb_t[h][:],
                                     start=(h == 0), stop=(h == NH - 1))
                for h in range(NH):
                    nc.tensor.matmul(psu[:], wus_bf[0][:, h, :], xtb_t[h][:],
                                     start=(h == 0), stop=(h == NH - 1))
                gs = actp.tile([I_SH, TC], F32, tag="gs", bufs=1)
                nc.scalar.activation(gs[:], psg[:], AF.Silu, bias=bgs_sb[:, 0:1])
                hs = hgep.tile([I_SH, TC], BF16, tag="hge_s")
                nc.vector.scalar_tensor_tensor(hs[:], psu[:], bus_sb[:, 0:1],
                                               gs[:], ALU.add, ALU.mult)

                # down projection, token-major output
                for j in range(NJ):
                    ts = t0 + j * P
                    out_sb = outp.tile([P, H], F32, tag="out")
                    for half in range(2):
                        hs0 = half * (H // 2)
                        pd = ps_d.tile([P, H // 2], F32, tag=f"d{half}",
                                       name=f"pd{half}")
                        m = 0
                        for e in range(E_LOC):
                            for i in range(NI):
                                nc.tensor.matmul(
                                    pd[:],
                                    hge[(e, i)][:, j * P:(j + 1) * P],
                                    wd_bf[e][:, i, hs0:hs0 + H // 2],
                                    start=(m == 0), stop=False)
                                m += 1
                        nc.tensor.matmul(pd[:],
                                         hs[:, j * P:(j + 1) * P],
                                         wds_bf[:, hs0:hs0 + H // 2],
                                         start=False, stop=False)
                        nc.tensor.matmul(pd[:],
                                         we_sb[:, ts:ts + P],
                                         bias5_sb[:, hs0:hs0 + H // 2],
                                         start=False, stop=True)
                        nc.vector.tensor_copy(out_sb[:, hs0:hs0 + H // 2], pd[:])
                    nc.scalar.dma_start(cc_in[ts:ts + P, :], out_sb[:])

            def reduce_chunk(ch):
                if ch != NCH - 1:
                    return
                nc.gpsimd.collective_compute(
                    "ReduceScatter",
                    ALU.add,
                    ins=[cc_in[:]],
                    outs=[cc_out[:]],
                    replica_groups=[list(range(NCORES))],
                )
                nc.scalar.dma_start(y_d[:], cc_out[:])

            # ---------- main loop ----------
            # Two-chunk-ahead x staging: Tile serializes DMA-transposes against
            # collectives (xbar-mode transitions), and RS(ch) only starts after
            # experts(ch) outputs land, so transposes must be issued two chunks
            # ahead to precede RS(ch) in queue order with slack. router(ch+2)
            # sits after experts(ch) in the PE's in-order stream so the PE never
            # waits on a not-yet-loaded residual tile.
            xtb_chunks[1] = stage_x(1)
            router(1, xtb_chunks[1])
            for ch in range(NCH):
                if ch + 2 < NCH:
                    xtb_chunks[ch + 2] = stage_x(ch + 2)
                experts(ch, xtb_chunks.pop(ch))
                if ch + 2 < NCH:
                    router(ch + 2, xtb_chunks[ch + 2])
                reduce_chunk(ch)

    nc.finalize()
    return nc


def prep_inputs(inputs):
    """Split/replicate full inputs into 8 per-core input maps (layout only)."""
    hs = np.ascontiguousarray(np.asarray(inputs["hidden_states"], dtype=np.float32))
    x = hs.reshape(T, H)
    Wr = np.asarray(inputs["Wr"], np.float32)
    br = np.asarray(inputs["br"], np.float32)
    Wg = np.asarray(inputs["Wg"], np.float32)
    bg = np.asarray(inputs["bg"], np.float32)
    Wu = np.asarray(inputs["Wu"], np.float32)
    bu = np.asarray(inputs["bu"], np.float32)
    Wd = np.asarray(inputs["Wd"], np.float32)
    bd = np.asarray(inputs["bd"], np.float32)
    Wg_s = np.asarray(inputs["Wg_s"], np.float32)
    bg_s = np.asarray(inputs["bg_s"], np.float32)
    Wu_s = np.asarray(inputs["Wu_s"], np.float32)
    bu_s = np.asarray(inputs["bu_s"], np.float32)
    Wd_s = np.asarray(inputs["Wd_s"], np.float32)
    bd_s = np.asarray(inputs["bd_s"], np.float32)

    in_maps = []
    for c in range(NCORES):
        loc = list(range(c * E_LOC, (c + 1) * E_LOC))
        rest = [e for e in range(E) if e not in loc]
        perm = loc + rest
        sh = slice(c * I_SH, (c + 1) * I_SH)
        bias5 = np.concatenate(
            [bd[loc], (bd_s if c == 0 else np.zeros_like(bd_s))[None, :]], axis=0)
        in_maps.append({
            "x": x,
            "wr": np.ascontiguousarray(Wr[:, perm]),
            "br": np.ascontiguousarray(br[perm]),
            "wg": np.ascontiguousarray(Wg[loc]),
            "wu": np.ascontiguousarray(Wu[loc]),
            "wd": np.ascontiguousarray(Wd[loc]),
            "bg": np.ascontiguousarray(bg[loc]),
            "bu": np.ascontiguousarray(bu[loc]),
            "bias5": np.ascontiguousarray(bias5),
            "wgs": np.ascontiguousarray(Wg_s[:, sh]),
            "wus": np.ascontiguousarray(Wu_s[:, sh]),
            "wds": np.ascontiguousarray(Wd_s[sh, :]),
            "bgs": np.ascontiguousarray(bg_s[sh]),
            "bus": np.ascontiguousarray(bu_s[sh]),
            "sel": np.ascontiguousarray(
                np.kron(np.eye(E_LOC, dtype=np.float32),
                        np.ones((1, P), dtype=np.float32))),
        })
    return in_maps


def assemble_output(results):
    """Reassemble [T, H]: single ReduceScatter gives core c rows [c*512:(c+1)*512]."""
    return np.concatenate([results[c]["y"] for c in range(NCORES)], axis=0)


_CACHE = {}


def get_runner():
    """Build + jit once; returns run(in_maps) -> list of per-core output dicts."""
    if "run" in _CACHE:
        return _CACHE["run"]
    import jax
    from jax.sharding import Mesh, PartitionSpec
    from jax.experimental.shard_map import shard_map
    from concourse import bass2jax

    nc = build_nc()
    bass2jax.install_neuronx_cc_hook()

    in_names = []
    out_names = []
    out_avals = []
    partition_name = nc.partition_id_tensor.name if nc.partition_id_tensor else None
    for alloc in nc.m.functions[0].allocations:
        if not isinstance(alloc, mybir.MemoryLocationSet):
            continue
        name = alloc.memorylocations[0].name
        if alloc.kind == "ExternalInput":
            if name != partition_name:
                in_names.append(name)
        elif alloc.kind == "ExternalOutput":
            out_names.append(name)
            out_avals.append(
                jax.core.ShapedArray(tuple(alloc.tensor_shape),
                                     mybir.dt.np(alloc.dtype)))
    n_params = len(in_names)
    n_outs = len(out_names)
    all_names = in_names + out_names + ([partition_name] if partition_name else [])
    donate = tuple(range(n_params, n_params + n_outs))

    def _body(*args):
        operands = list(args)
        if partition_name is not None:
            operands.append(bass2jax.partition_id_tensor())
        return tuple(bass2jax._bass_exec_p.bind(
            *operands,
            out_avals=tuple(out_avals),
            in_names=tuple(all_names),
            out_names=tuple(out_names),
            lowering_input_output_aliases=(),
            sim_require_finite=True,
            sim_require_nnan=True,
            nc=nc,
        ))

    devices = jax.devices()[:NCORES]
    mesh = Mesh(np.asarray(devices), ("core",))
    in_specs = (PartitionSpec("core"),) * (n_params + n_outs)
    out_specs = (PartitionSpec("core"),) * n_outs
    sharded = jax.jit(
        shard_map(_body, mesh=mesh, in_specs=in_specs, out_specs=out_specs,
                  check_rep=False),
        donate_argnums=donate, keep_unused=True)

    def run(in_maps, dev_inputs=None):
        if dev_inputs is None:
            dev_inputs = [
                np.concatenate([np.asarray(in_maps[c][n]) for c in range(NCORES)],
                               axis=0)
                for n in in_names
            ]
        zeros = [np.zeros((NCORES * a.shape[0], *a.shape[1:]), a.dtype)
                 for a in out_avals]
        outs = sharded(*dev_inputs, *zeros)
        return [
            {name: np.asarray(outs[i]).reshape(NCORES, *out_avals[i].shape)[c]
             for i, name in enumerate(out_names)}
            for c in range(NCORES)
        ]

    _CACHE["run"] = run
    _CACHE["meta"] = (in_names, out_names, out_avals, sharded, mesh)
    return run


def kernel(**inputs) -> np.ndarray:
    run = get_runner()
    in_maps = prep_inputs(inputs)
    results = run(in_maps)
    return assemble_output(results).reshape(B, S, H).astype(np.float32)

